# revision 1
# baseline (speedup 1.0000x reference)
"""Trainium2 Bass kernel for nn_Decoder (ragged LSTM decoder), 8-core SPMD.

Strategy: data-parallel over batch (16 batch rows per core). Per core:
  Phase A (parallel over t): ragged word-avg via banded matmul against
    on-device-built coefficient matrices; pos-embedding gather via one-hot
    matmul; z = tanh(x @ combine_W.T + b); G_in = z @ W_ih.T + biases -> DRAM.
    Also produces enc.T tiles (via identity matmul) -> DRAM for phase C.
  Phase B (sequential scan, T=512): g = G_in[t] + h @ W_hh.T (36 128x128
    matmuls/step, gates packed [128 part, 12*16 cols]); LSTM cell on ACT/DVE;
    h2 history -> DRAM.
  Phase C (parallel): logits = [h2, e_t] @ out_W.T in [cols, label] layout,
    t==0 appID fixup, log_softmax along free axis, DMA out.
Column order everywhere is t-major: col = t*16 + b_local.
"""
import sys
sys.path.insert(0, "/opt/trn_rl_repo")

import numpy as np

B, T, H = 128, 512, 384
D_ENC, HID = 768, 768
POS_SIZE, POS_DIM, LABEL = 64, 128, 128
APP_ID = 3
NCORES = 8
BC = B // NCORES          # 16 batch rows per core
COLS = T * BC             # 8192 (t-major)
NTT = T // 128            # 4 t-tiles
F32 = None                # set after mybir import

_COMPILED = None


def _build(reps=1, phases='abc'):
    import concourse.bass as bass
    import concourse.mybir as mybir
    import concourse.tile as tile
    from concourse import bacc
    from contextlib import ExitStack

    f32 = mybir.dt.float32
    AF = mybir.ActivationFunctionType
    ALU = mybir.AluOpType

    nc = bacc.Bacc(None, target_bir_lowering=False, debug=False,
                   num_devices=NCORES)

    def param(name, shape):
        return nc.declare_dram_parameter(name, list(shape), f32, isOutput=False)

    enc = param("enc", [BC, T, D_ENC])
    sreld = param("sreld", [BC, T])
    srelc = param("srelc", [BC, T])
    recipv = param("recipv", [BC, T])
    pidcol = param("pidcol", [COLS])
    combWT = param("combWT", [7, 128, HID])        # combine_W.T split on K
    wihT = param("wihT", [6, 128, 4 * H])
    whhT = param("whhT", [3, 128, 4 * H])
    outWhT = param("outWhT", [3, 128, LABEL])
    outWeT = param("outWeT", [6, 128, LABEL])
    posw = param("posw", [POS_SIZE, POS_DIM])
    combb = param("combb", [6, 128])
    biassum = param("biassum", [12, 128])
    mlt = param("mlt", [128, 128])                  # [p,t] = 1.0 if p<t
    iota = param("iota", [128])
    id128 = param("id128", [128, 128])

    out = nc.declare_dram_parameter("out", [BC, T, LABEL], f32, isOutput=True)

    encT_d = nc.dram_tensor("encT_d", [6, 128, T, BC], f32)
    gin_d = nc.dram_tensor("gin_d", [128, T, 12, BC], f32)
    h2_d = nc.dram_tensor("h2_d", [128, 3, T * BC], f32)

    with tile.TileContext(nc) as tc, ExitStack() as top:
        singles = top.enter_context(tc.tile_pool(name="singles", bufs=1))

        # ---- resident weights/constants in SBUF ----
        combWT_sb = singles.tile([128, 7, HID], f32)
        wihT_sb = singles.tile([128, 6, 4 * H], f32)
        whhT_sb = singles.tile([128, 3, 4 * H], f32)
        outWhT_sb = singles.tile([128, 3, LABEL], f32)
        outWeT_sb = singles.tile([128, 6, LABEL], f32)
        posw_sb = singles.tile([POS_SIZE, POS_DIM], f32)
        combb_sb = singles.tile([128, 6], f32)
        biassum_sb = singles.tile([128, 12], f32)
        mlt_sb = singles.tile([128, 128], f32)
        iota_sb = singles.tile([128, 1], f32)
        id_sb = singles.tile([128, 128], f32)
        nc.sync.dma_start(out=combWT_sb, in_=combWT.ap().rearrange("k p m -> p k m"))
        nc.sync.dma_start(out=wihT_sb, in_=wihT.ap().rearrange("k p m -> p k m"))
        nc.sync.dma_start(out=whhT_sb, in_=whhT.ap().rearrange("k p m -> p k m"))
        nc.sync.dma_start(out=outWhT_sb, in_=outWhT.ap().rearrange("k p m -> p k m"))
        nc.sync.dma_start(out=outWeT_sb, in_=outWeT.ap().rearrange("k p m -> p k m"))
        nc.sync.dma_start(out=posw_sb, in_=posw.ap())
        nc.sync.dma_start(out=combb_sb, in_=combb.ap().rearrange("m p -> p m"))
        nc.sync.dma_start(out=biassum_sb, in_=biassum.ap().rearrange("m p -> p m"))
        nc.sync.dma_start(out=mlt_sb, in_=mlt.ap())
        nc.sync.dma_start(out=iota_sb, in_=iota.ap().rearrange("(p o) -> p o", o=1))
        nc.sync.dma_start(out=id_sb, in_=id128.ap())

        for _rep in range(reps):
            # ================= Phase A =================
            with ExitStack() as pa:
              if 'a' in phases:
                xt_pool = pa.enter_context(tc.tile_pool(name="xt", bufs=1))
                a_pool = pa.enter_context(tc.tile_pool(name="awork", bufs=2))
                bc_pool = pa.enter_context(tc.tile_pool(name="abcast", bufs=3))
                z_pool = pa.enter_context(tc.tile_pool(name="zt", bufs=2))
                g_pool = pa.enter_context(tc.tile_pool(name="ginw", bufs=3))
                ps1 = pa.enter_context(tc.tile_pool(name="psA1", bufs=2, space="PSUM"))
                psp = pa.enter_context(tc.tile_pool(name="psPos", bufs=1, space="PSUM"))
                ps3 = pa.enter_context(tc.tile_pool(name="psA3", bufs=2, space="PSUM"))
                ps4 = pa.enter_context(tc.tile_pool(name="psA4", bufs=2, space="PSUM"))

                for tt in range(NTT):
                    t0 = tt * 128
                    xT = xt_pool.tile([128, 7, 128, BC], f32)  # [p, ktile, t, b]

                    # --- A2: pos embedding via one-hot matmul ---
                    for c4 in range(4):
                        colb = t0 * BC + c4 * 512
                        pid_b = a_pool.tile([POS_SIZE, 512], f32, tag="pidb")
                        src = bass.AP(tensor=pidcol.ap().tensor,
                                      offset=pidcol.ap().offset + colb,
                                      ap=[[0, POS_SIZE], [1, 512]])
                        nc.gpsimd.dma_start(out=pid_b, in_=src)
                        oh = a_pool.tile([POS_SIZE, 512], f32, tag="oh")
                        nc.vector.tensor_scalar(oh, pid_b, iota_sb[0:POS_SIZE, :],
                                                None, ALU.is_equal)
                        pp = psp.tile([128, 512], f32, tag="pspos")
                        nc.tensor.matmul(pp, posw_sb, oh, start=True, stop=True)
                        nc.scalar.activation(
                            xT[:, 0, c4 * 32:(c4 + 1) * 32, :], pp, AF.Copy)

                    # --- A1: ragged word-average + enc transpose ---
                    for b in range(BC):
                        enc_sb = a_pool.tile([128, D_ENC], f32, tag="encin")
                        nc.sync.dma_start(out=enc_sb, in_=enc.ap()[b, t0:t0 + 128, :])
                        if tt > 0:
                            encpre = a_pool.tile([8, D_ENC], f32, tag="encpre")
                            nc.sync.dma_start(out=encpre,
                                              in_=enc.ap()[b, t0 - 8:t0, :])
                        sreld_b = bc_pool.tile([128, 128], f32, tag="sreldb")
                        recip_b = bc_pool.tile([128, 128], f32, tag="recipb")
                        nc.gpsimd.dma_start(
                            out=sreld_b,
                            in_=bass.AP(tensor=sreld.ap().tensor,
                                        offset=sreld.ap().offset + b * T + t0,
                                        ap=[[0, 128], [1, 128]]))
                        nc.gpsimd.dma_start(
                            out=recip_b,
                            in_=bass.AP(tensor=recipv.ap().tensor,
                                        offset=recipv.ap().offset + b * T + t0,
                                        ap=[[0, 128], [1, 128]]))
                        C = bc_pool.tile([128, 128], f32, tag="cmat")
                        nc.vector.scalar_tensor_tensor(
                            C, sreld_b, iota_sb, mlt_sb, ALU.is_le, ALU.mult)
                        nc.vector.tensor_mul(C, C, recip_b)
                        if tt > 0:
                            srelc_b = bc_pool.tile([8, 128], f32, tag="srelcb")
                            nc.gpsimd.dma_start(
                                out=srelc_b,
                                in_=bass.AP(tensor=srelc.ap().tensor,
                                            offset=srelc.ap().offset + b * T + t0,
                                            ap=[[0, 8], [1, 128]]))
                            Ccr = bc_pool.tile([8, 128], f32, tag="ccr")
                            nc.vector.scalar_tensor_tensor(
                                Ccr, srelc_b, iota_sb[0:8, :], recip_b[0:8, :],
                                ALU.is_le, ALU.mult)
                        for fc in range(6):
                            ps = ps1.tile([128, 256], f32, tag="psw")
                            lhs = enc_sb[:, fc * 128:(fc + 1) * 128]
                            nc.tensor.matmul(ps[:, 0:128], lhs, C,
                                             start=True, stop=(tt == 0))
                            if tt > 0:
                                nc.tensor.matmul(
                                    ps[:, 0:128],
                                    encpre[:, fc * 128:(fc + 1) * 128], Ccr,
                                    start=False, stop=True)
                            nc.tensor.matmul(ps[:, 128:256], lhs, id_sb,
                                             start=True, stop=True)
                            nc.vector.tensor_copy(xT[:, 1 + fc, :, b], ps[:, 0:128])
                            ecp = a_pool.tile([128, 128], f32, tag="ecp")
                            nc.scalar.activation(ecp, ps[:, 128:256], AF.Copy)
                            nc.sync.dma_start(out=encT_d.ap()[fc, :, t0:t0 + 128, b],
                                              in_=ecp)

                    # --- A3 + A4 per 512-col chunk ---
                    for c4 in range(4):
                        tg = t0 + c4 * 32
                        zT = z_pool.tile([128, 6, 512], f32)
                        for m in range(6):
                            ps = ps3.tile([128, 512], f32)
                            for k in range(7):
                                nc.tensor.matmul(
                                    ps, combWT_sb[:, k, m * 128:(m + 1) * 128],
                                    xT[:, k, c4 * 32:(c4 + 1) * 32, :],
                                    start=(k == 0), stop=(k == 6))
                            nc.scalar.activation(zT[:, m, :], ps, AF.Tanh,
                                                 bias=combb_sb[:, m:m + 1])
                        if tt == 0 and c4 == 0:
                            for m in range(6):
                                nc.vector.memset(zT[:, m, 0:BC], 0.0)
                        for j in range(12):
                            ps = ps4.tile([128, 512], f32)
                            for k in range(6):
                                nc.tensor.matmul(
                                    ps, wihT_sb[:, k, j * 128:(j + 1) * 128],
                                    zT[:, k, :], start=(k == 0), stop=(k == 5))
                            gsb = g_pool.tile([128, 32, BC], f32)
                            nc.scalar.activation(gsb, ps, AF.Identity,
                                                 bias=biassum_sb[:, j:j + 1])
                            nc.sync.dma_start(out=gin_d.ap()[:, tg:tg + 32, j, :],
                                              in_=gsb)

            # ================= Phase B: scan =================
            with ExitStack() as pb:
              if 'b' in phases:
                  ginr = pb.enter_context(tc.tile_pool(name="ginr", bufs=3))
                  hpool = pb.enter_context(tc.tile_pool(name="hp", bufs=3))
                  cpool = pb.enter_context(tc.tile_pool(name="cp", bufs=3))
                  gapool = pb.enter_context(tc.tile_pool(name="ga", bufs=3))
                  tmp = pb.enter_context(tc.tile_pool(name="stmp", bufs=6))
                  pss = pb.enter_context(tc.tile_pool(name="psS", bufs=2, space="PSUM"))

                  hT = hpool.tile([128, 3 * BC], f32, tag="h")
                  cT = cpool.tile([128, BC * 3], f32, tag="c")
                  nc.vector.memset(hT, 0.0)
                  nc.vector.memset(cT, 0.0)
                  SFUNC = [AF.Sigmoid, AF.Sigmoid, AF.Tanh, AF.Sigmoid]
                  for blk in range(T // 16):
                      gch = ginr.tile([128, 16, 12, BC], f32)
                      nc.sync.dma_start(out=gch,
                                        in_=gin_d.ap()[:, blk * 16:(blk + 1) * 16, :, :])
                      for s in range(16):
                          t = blk * 16 + s
                          psg = [pss.tile([128, 3 * BC], f32, tag=f"psg{gi}",
                                          name=f"psg{gi}")
                                 for gi in range(4)]
                          for gi in range(4):
                              for jj in range(3):
                                  j = gi * 3 + jj
                                  for k in range(3):
                                      nc.tensor.matmul(
                                          psg[gi][:, jj * BC:(jj + 1) * BC],
                                          whhT_sb[:, k, j * 128:(j + 1) * 128],
                                          hT[:, k * BC:(k + 1) * BC],
                                          start=(k == 0), stop=(k == 2))
                          gact = gapool.tile([128, 12 * BC], f32)
                          for gi in range(4):
                              gs = tmp.tile([128, 3 * BC], f32, tag="gs")
                              nc.vector.tensor_add(
                                  gs, psg[gi], gch[:, s, gi * 3:(gi + 1) * 3, :])
                              nc.scalar.activation(
                                  gact[:, gi * 3 * BC:(gi + 1) * 3 * BC], gs,
                                  SFUNC[gi])
                          i_a = gact[:, 0:3 * BC]
                          f_a = gact[:, 3 * BC:6 * BC]
                          g_a = gact[:, 6 * BC:9 * BC]
                          o_a = gact[:, 9 * BC:12 * BC]
                          t1 = tmp.tile([128, 3 * BC], f32, tag="t1")
                          nc.vector.tensor_mul(t1, f_a, cT)
                          t2 = tmp.tile([128, 3 * BC], f32, tag="t2")
                          nc.vector.tensor_mul(t2, i_a, g_a)
                          cT = cpool.tile([128, 3 * BC], f32, tag="c")
                          nc.vector.tensor_add(cT, t1, t2)
                          tc2 = tmp.tile([128, 3 * BC], f32, tag="tc2")
                          nc.scalar.activation(tc2, cT, AF.Tanh)
                          hT = hpool.tile([128, 3 * BC], f32, tag="h")
                          nc.vector.tensor_mul(hT, o_a, tc2)
                          nc.sync.dma_start(out=h2_d.ap()[:, :, t * BC:(t + 1) * BC], in_=hT)

            # ================= Phase C: logits + log_softmax =================
            with ExitStack() as pc:
              if 'c' in phases:
                  h2r = pc.enter_context(tc.tile_pool(name="h2r", bufs=3))
                  encr = pc.enter_context(tc.tile_pool(name="encr", bufs=3))
                  smp = pc.enter_context(tc.tile_pool(name="smp", bufs=4))
                  smc = pc.enter_context(tc.tile_pool(name="smc", bufs=6))
                  psc = pc.enter_context(tc.tile_pool(name="psC", bufs=4, space="PSUM"))

                  for ch in range(COLS // 128):
                      tc0 = ch * 8
                      h2t = h2r.tile([128, 3, 128], f32)
                      nc.sync.dma_start(out=h2t,
                                        in_=h2_d.ap()[:, :, ch * 128:(ch + 1) * 128])
                      enct = encr.tile([128, 6, 8, BC], f32)
                      for fc in range(6):
                          nc.sync.dma_start(out=enct[:, fc, :, :],
                                            in_=encT_d.ap()[fc, :, tc0:tc0 + 8, :])
                      ps = psc.tile([128, LABEL], f32)
                      for k in range(3):
                          nc.tensor.matmul(ps, h2t[:, k, :], outWhT_sb[:, k, :],
                                           start=(k == 0), stop=False)
                      for fc in range(6):
                          nc.tensor.matmul(ps, enct[:, fc, :, :],
                                           outWeT_sb[:, fc, :],
                                           start=False, stop=(fc == 5))
                      lg = smp.tile([128, LABEL], f32, tag="lg")
                      nc.vector.tensor_copy(lg, ps)
                      if ch == 0:
                          nc.vector.memset(lg[0:BC, APP_ID:APP_ID + 1], -1e10)
                      mx = smc.tile([128, 1], f32, tag="mx")
                      nc.vector.tensor_reduce(mx, lg, mybir.AxisListType.X, ALU.max)
                      xm = smp.tile([128, LABEL], f32, tag="xm")
                      nc.vector.tensor_scalar(xm, lg, mx, None, ALU.subtract)
                      et = smp.tile([128, LABEL], f32, tag="et")
                      ssum = smc.tile([128, 1], f32, tag="ssum")
                      nc.scalar.activation(et, xm, AF.Exp, accum_out=ssum)
                      lns = smc.tile([128, 1], f32, tag="lns")
                      nc.scalar.activation(lns, ssum, AF.Ln)
                      res = smp.tile([128, LABEL], f32, tag="res")
                      nc.vector.tensor_scalar(res, xm, lns, None, ALU.subtract)
                      nc.sync.dma_start(
                          out=out.ap().rearrange("b t l -> t b l")[tc0:tc0 + 8, :, :],
                          in_=res)

    nc.compile()
    return nc


def _host_prep(encoder_out, pos_embed_w, W_ih, W_hh, b_ih, b_hh,
               combine_W, combine_b, out_W, word_start, pos_ids):
    enc = np.ascontiguousarray(np.asarray(encoder_out, dtype=np.float32))
    ws = np.asarray(word_start)
    pid = np.asarray(pos_ids)
    tgrid = np.arange(T)[:, None]
    valid = ws >= 0
    s = np.clip(ws, 0, None)
    ln = np.maximum(tgrid - s, 1)
    recipv = (valid / ln).astype(np.float32)
    t0 = (tgrid // 128) * 128
    sreld = (s - t0).astype(np.float32)
    srelc = (s - t0 + 8).astype(np.float32)

    shared = dict(
        combWT=np.ascontiguousarray(
            np.asarray(combine_W, np.float32).T).reshape(7, 128, HID),
        wihT=np.ascontiguousarray(
            np.asarray(W_ih, np.float32).T).reshape(6, 128, 4 * H),
        whhT=np.ascontiguousarray(
            np.asarray(W_hh, np.float32).T).reshape(3, 128, 4 * H),
        outWhT=np.ascontiguousarray(
            np.asarray(out_W, np.float32)[:, :H].T).reshape(3, 128, LABEL),
        outWeT=np.ascontiguousarray(
            np.asarray(out_W, np.float32)[:, H:].T).reshape(6, 128, LABEL),
        posw=np.asarray(pos_embed_w, np.float32),
        combb=np.asarray(combine_b, np.float32).reshape(6, 128),
        biassum=(np.asarray(b_ih, np.float32)
                 + np.asarray(b_hh, np.float32)).reshape(12, 128),
        mlt=(np.arange(128)[:, None] < np.arange(128)[None, :]
             ).astype(np.float32),
        iota=np.arange(128, dtype=np.float32),
        id128=np.eye(128, dtype=np.float32),
    )
    in_maps = []
    for c in range(NCORES):
        bs = slice(c * BC, (c + 1) * BC)
        m = dict(shared)
        m["enc"] = np.ascontiguousarray(enc[bs])
        m["sreld"] = np.ascontiguousarray(sreld[:, bs].T)
        m["srelc"] = np.ascontiguousarray(srelc[:, bs].T)
        m["recipv"] = np.ascontiguousarray(recipv[:, bs].T)
        m["pidcol"] = np.ascontiguousarray(
            pid[:, bs].astype(np.float32).reshape(-1))
        in_maps.append(m)
    return in_maps


def _get_compiled():
    global _COMPILED
    if _COMPILED is None:
        import os
        reps = int(os.environ.get("BK_REPS", "1"))
        phases = os.environ.get("BK_PHASES", "abc")
        _COMPILED = _build(reps=reps, phases=phases)
    return _COMPILED


def kernel(**inputs):
    from concourse.bass_utils import run_bass_kernel_spmd
    nc = _get_compiled()
    in_maps = _host_prep(**inputs)
    res = run_bass_kernel_spmd(nc, in_maps, list(range(NCORES)))
    outs = [res.results[c]["out"] for c in range(NCORES)]
    full = np.concatenate(outs, axis=0)           # [B, T, LABEL]
    return full.reshape(B * T, LABEL).astype(np.float32)



# revision 17
# speedup vs baseline: 9.9556x; 9.9556x over previous
"""Trainium2 Bass kernel for nn_Decoder (ragged LSTM decoder), 8-core SPMD.

v2 strategy (vs v1 baseline):
  * bf16 matmuls everywhere (fp32 matmul is 4 cyc/row on TRN2; bf16 is 1
    and gets 2x fast-weight-load). Elementwise state stays fp32.
  * All DRAM layouts are contiguous per-partition (>=512B runs); the v1
    kernel issued ~7.4M 4-64B DMA descriptors (enc transpose scatter,
    strided gin/h2 writes) which made it DMA-descriptor-bound.
  * enc transpose for phase C is done on the host (pure layout prep).
  * Scan adds g_in via an identity-matmul accumulate into PSUM (saves 4
    DVE adds/step), sigmoid/tanh ACTs read PSUM directly.

Data-parallel over batch: 16 rows per core. Column order is t-major:
col = t*16 + b_local. Gate order i,f,g,o; j = 128-wide gate chunk (0-11),
k = 128-wide h chunk (0-2).
"""
import sys
sys.path.insert(0, "/opt/trn_rl_repo")

import numpy as np
import ml_dtypes

B, T, H = 128, 512, 384
D_ENC, HID = 768, 768
POS_SIZE, POS_DIM, LABEL = 64, 128, 128
APP_ID = 3
NCORES = 8
BC = B // NCORES          # 16 batch rows per core
COLS = T * BC             # 8192 (t-major)
NTT = T // 128            # 4 t-tiles
NBLK = T // 16            # 32 scan blocks

_COMPILED = None


def _build(reps=1, phases='abc'):
    import concourse.bass as bass
    import concourse.mybir as mybir
    import concourse.tile as tile
    from concourse import bacc
    from contextlib import ExitStack

    f32 = mybir.dt.float32
    bf16 = mybir.dt.bfloat16
    AF = mybir.ActivationFunctionType
    ALU = mybir.AluOpType

    nc = bacc.Bacc(None, target_bir_lowering=False, debug=False,
                   num_devices=NCORES)

    def param(name, shape, dt=f32):
        return nc.declare_dram_parameter(name, list(shape), dt, isOutput=False)

    enc = param("enc", [BC, T, D_ENC], bf16)
    encTd = param("encTd", [COLS // 128, 128, 6 * 8 * BC], bf16)
    sreld = param("sreld", [BC, T])
    srelc = param("srelc", [BC, T])
    recipv = param("recipv", [BC, T])
    pidcol = param("pidcol", [COLS])
    combWT = param("combWT", [7, 128, HID], bf16)      # combine_W.T split on K
    wihT = param("wihT", [6, 128, 4 * H], bf16)
    whhT = param("whhT", [3, 128, 4 * H], bf16)
    outWhT = param("outWhT", [3, 128, LABEL], bf16)
    outWeT = param("outWeT", [6, 128, LABEL], bf16)
    posw = param("posw", [POS_SIZE, POS_DIM], bf16)
    combb = param("combb", [6, 128])
    biassum = param("biassum", [12, 128])
    mlt = param("mlt", [128, 128])                      # [p,t] = 1.0 if p<t
    iota = param("iota", [128])
    id128 = param("id128", [128, 128], bf16)

    out = nc.declare_dram_parameter("out", [BC, T, LABEL], f32, isOutput=True)

    # (p, blk, j, t16, b) and (p, blk, s, k, b) — contiguous per partition
    gin_d = nc.dram_tensor("gin_d", [128, NBLK, 12, 16, BC], bf16)
    h2_d = nc.dram_tensor("h2_d", [128, NBLK, 3, 16, BC], bf16)

    with tile.TileContext(nc) as tc, ExitStack() as top:
        singles = top.enter_context(tc.tile_pool(name="singles", bufs=1))

        # ---- resident weights/constants in SBUF ----
        combWT_sb = singles.tile([128, 7, HID], bf16)
        wihT_sb = singles.tile([128, 6, 4 * H], bf16)
        whhT_sb = singles.tile([128, 3, 4 * H], bf16)
        outWhT_sb = singles.tile([128, 3, LABEL], bf16)
        outWeT_sb = singles.tile([128, 6, LABEL], bf16)
        posw_sb = singles.tile([POS_SIZE, POS_DIM], bf16)
        combb_sb = singles.tile([128, 6], f32)
        biassum_sb = singles.tile([128, 12], f32)
        mlt_sb = singles.tile([128, 128], f32)
        iota_sb = singles.tile([128, 1], f32)
        id_sb = singles.tile([128, 128], bf16)
        nc.sync.dma_start(out=combWT_sb, in_=combWT.ap().rearrange("k p m -> p k m"))
        nc.sync.dma_start(out=wihT_sb, in_=wihT.ap().rearrange("k p m -> p k m"))
        nc.sync.dma_start(out=whhT_sb, in_=whhT.ap().rearrange("k p m -> p k m"))
        nc.sync.dma_start(out=outWhT_sb, in_=outWhT.ap().rearrange("k p m -> p k m"))
        nc.sync.dma_start(out=outWeT_sb, in_=outWeT.ap().rearrange("k p m -> p k m"))
        nc.sync.dma_start(out=posw_sb, in_=posw.ap())
        nc.sync.dma_start(out=combb_sb, in_=combb.ap().rearrange("m p -> p m"))
        nc.sync.dma_start(out=biassum_sb, in_=biassum.ap().rearrange("m p -> p m"))
        nc.sync.dma_start(out=mlt_sb, in_=mlt.ap())
        nc.sync.dma_start(out=iota_sb, in_=iota.ap().rearrange("(p o) -> p o", o=1))
        nc.sync.dma_start(out=id_sb, in_=id128.ap())

        for _rep in range(reps):
            # ================= Phase A =================
            with ExitStack() as pa:
              if 'a' in phases:
                enc_pool = pa.enter_context(tc.tile_pool(name="encp", bufs=2))
                xt_pool = pa.enter_context(tc.tile_pool(name="xt", bufs=2))
                a_pool = pa.enter_context(tc.tile_pool(name="awork", bufs=2))
                bc_pool = pa.enter_context(tc.tile_pool(name="abcast", bufs=2))
                z_pool = pa.enter_context(tc.tile_pool(name="zt", bufs=2))
                g_pool = pa.enter_context(tc.tile_pool(name="ginw", bufs=2))
                psp = pa.enter_context(tc.tile_pool(name="psPos", bufs=1, space="PSUM"))
                ps1 = pa.enter_context(tc.tile_pool(name="psA1", bufs=2, space="PSUM"))
                ps3 = pa.enter_context(tc.tile_pool(name="psA3", bufs=2, space="PSUM"))
                ps4 = pa.enter_context(tc.tile_pool(name="psA4", bufs=2, space="PSUM"))

                enc_prev = None
                for tt in range(NTT):
                    t0 = tt * 128
                    enc_sb = enc_pool.tile([128, BC, D_ENC], bf16, tag="enc")
                    nc.sync.dma_start(
                        out=enc_sb,
                        in_=enc.ap()[:, t0:t0 + 128, :].rearrange("b t f -> t b f"))
                    xT = xt_pool.tile([128, 7, 128, BC], bf16)  # [p, k, t, b]

                    # --- A2: pos embedding via one-hot matmul ---
                    for c4 in range(4):
                        colb = t0 * BC + c4 * 512
                        pid_b = a_pool.tile([POS_SIZE, 512], f32, tag="pidb")
                        src = bass.AP(tensor=pidcol.ap().tensor,
                                      offset=pidcol.ap().offset + colb,
                                      ap=[[0, POS_SIZE], [1, 512]])
                        nc.gpsimd.dma_start(out=pid_b, in_=src)
                        oh = a_pool.tile([POS_SIZE, 512], bf16, tag="oh")
                        nc.vector.tensor_scalar(oh, pid_b, iota_sb[0:POS_SIZE, :],
                                                None, ALU.is_equal)
                        pp = psp.tile([128, 512], f32, tag="pspos")
                        nc.tensor.matmul(pp, posw_sb, oh, start=True, stop=True)
                        nc.scalar.activation(
                            xT[:, 0, c4 * 32:(c4 + 1) * 32, :], pp, AF.Copy)

                    # --- A1: ragged word-average ---
                    for b in range(BC):
                        sreld_b = bc_pool.tile([128, 128], f32, tag="sreldb")
                        recip_b = bc_pool.tile([128, 128], f32, tag="recipb")
                        nc.gpsimd.dma_start(
                            out=sreld_b,
                            in_=bass.AP(tensor=sreld.ap().tensor,
                                        offset=sreld.ap().offset + b * T + t0,
                                        ap=[[0, 128], [1, 128]]))
                        nc.gpsimd.dma_start(
                            out=recip_b,
                            in_=bass.AP(tensor=recipv.ap().tensor,
                                        offset=recipv.ap().offset + b * T + t0,
                                        ap=[[0, 128], [1, 128]]))
                        C0 = bc_pool.tile([128, 128], f32, tag="c0")
                        nc.vector.scalar_tensor_tensor(
                            C0, sreld_b, iota_sb, mlt_sb, ALU.is_le, ALU.mult)
                        Cb = bc_pool.tile([128, 128], bf16, tag="cmat")
                        nc.vector.tensor_mul(Cb, C0, recip_b)
                        if tt > 0:
                            encpre = bc_pool.tile([8, D_ENC], bf16, tag="encpre")
                            nc.sync.dma_start(out=encpre,
                                              in_=enc.ap()[b, t0 - 8:t0, :])
                            srelc_b = bc_pool.tile([8, 128], f32, tag="srelcb")
                            nc.gpsimd.dma_start(
                                out=srelc_b,
                                in_=bass.AP(tensor=srelc.ap().tensor,
                                            offset=srelc.ap().offset + b * T + t0,
                                            ap=[[0, 8], [1, 128]]))
                            Ccr = bc_pool.tile([8, 128], bf16, tag="ccr")
                            nc.vector.scalar_tensor_tensor(
                                Ccr, srelc_b, iota_sb[0:8, :], recip_b[0:8, :],
                                ALU.is_le, ALU.mult)
                        for g3 in range(2):          # two groups of 3 f-chunks
                            ps = ps1.tile([128, 3, 128], f32, tag="psw")
                            for fi in range(3):
                                fc = g3 * 3 + fi
                                lhs = enc_sb[:, b, fc * 128:(fc + 1) * 128]
                                nc.tensor.matmul(ps[:, fi, :], lhs, Cb,
                                                 start=True, stop=(tt == 0))
                                if tt > 0:
                                    nc.tensor.matmul(
                                        ps[:, fi, :],
                                        encpre[:, fc * 128:(fc + 1) * 128],
                                        Ccr, start=False, stop=True)
                            dst = xT[:, 1 + g3 * 3:4 + g3 * 3, :, b]
                            if b % 2 == 0:
                                nc.vector.tensor_copy(dst, ps)
                            else:
                                nc.scalar.activation(dst, ps, AF.Copy)

                    # --- A3 + A4 per 512-col chunk ---
                    for c4 in range(4):
                        zT = z_pool.tile([128, 6, 512], bf16)
                        for m in range(6):
                            ps = ps3.tile([128, 512], f32)
                            for k in range(7):
                                nc.tensor.matmul(
                                    ps, combWT_sb[:, k, m * 128:(m + 1) * 128],
                                    xT[:, k, c4 * 32:(c4 + 1) * 32, :],
                                    start=(k == 0), stop=(k == 6))
                            nc.scalar.activation(zT[:, m, :], ps, AF.Tanh,
                                                 bias=combb_sb[:, m:m + 1])
                        if tt == 0 and c4 == 0:
                            nc.vector.memset(zT[:, :, 0:BC], 0.0)
                        stage = g_pool.tile([128, 2, 12, 16, BC], bf16)
                        for j in range(12):
                            ps = ps4.tile([128, 512], f32)
                            for k in range(6):
                                nc.tensor.matmul(
                                    ps, wihT_sb[:, k, j * 128:(j + 1) * 128],
                                    zT[:, k, :], start=(k == 0), stop=(k == 5))
                            nc.scalar.activation(stage[:, :, j, :, :], ps,
                                                 AF.Identity,
                                                 bias=biassum_sb[:, j:j + 1])
                        blk0 = tt * 8 + c4 * 2
                        nc.sync.dma_start(out=gin_d.ap()[:, blk0:blk0 + 2],
                                          in_=stage)
                    enc_prev = enc_sb

            # ================= Phase B: scan =================
            with ExitStack() as pb:
              if 'b' in phases:
                  ginr = pb.enter_context(tc.tile_pool(name="ginr", bufs=3))
                  hpool = pb.enter_context(tc.tile_pool(name="hp", bufs=2))
                  cpool = pb.enter_context(tc.tile_pool(name="cp", bufs=3))
                  spool = pb.enter_context(tc.tile_pool(name="sp", bufs=3))
                  pss = pb.enter_context(tc.tile_pool(name="psS", bufs=2, space="PSUM"))

                  h0 = spool.tile([128, 3 * BC], bf16, tag="h0")
                  nc.vector.memset(h0, 0.0)
                  cT = cpool.tile([128, 3 * BC], f32, tag="c")
                  nc.vector.memset(cT, 0.0)
                  h2prev = None
                  for blk in range(NBLK):
                      gch = ginr.tile([128, 12, 16, BC], bf16)
                      nc.sync.dma_start(out=gch, in_=gin_d.ap()[:, blk])
                      h2blk = hpool.tile([128, 3, 16, BC], bf16)  # (k, s, b)
                      for s in range(16):
                          t = blk * 16 + s
                          if t == 0:
                              rhs_ks = [h0[:, k * BC:(k + 1) * BC]
                                        for k in range(3)]
                          elif s == 0:
                              rhs_ks = [h2prev[:, k, 15, :] for k in range(3)]
                          else:
                              rhs_ks = [h2blk[:, k, s - 1, :] for k in range(3)]
                          psg = [pss.tile([128, 3, BC], f32, tag=f"psg{gi}",
                                          name=f"psg{gi}")
                                 for gi in range(4)]
                          for gi in range(4):
                              for jj in range(3):
                                  j = gi * 3 + jj
                                  for k in range(3):
                                      nc.tensor.matmul(
                                          psg[gi][:, jj, :],
                                          whhT_sb[:, k, j * 128:(j + 1) * 128],
                                          rhs_ks[k],
                                          start=(k == 0), stop=(k == 2))
                          gsum = spool.tile([128, 192], f32, tag="gsum")
                          for gi in range(4):
                              nc.vector.tensor_add(
                                  gsum[:, gi * 48:(gi + 1) * 48], psg[gi],
                                  gch[:, gi * 3:(gi + 1) * 3, s, :])
                          sfi = spool.tile([128, 96], f32, tag="sfi")
                          nc.scalar.activation(sfi, gsum[:, 0:96], AF.Sigmoid)
                          tg = spool.tile([128, 48], f32, tag="tg")
                          nc.scalar.activation(tg, gsum[:, 96:144], AF.Tanh)
                          so = spool.tile([128, 48], f32, tag="so")
                          nc.scalar.activation(so, gsum[:, 144:192], AF.Sigmoid)
                          ta = spool.tile([128, 48], f32, tag="ta")
                          nc.vector.tensor_mul(ta, sfi[:, 48:96], cT)
                          tb = spool.tile([128, 48], f32, tag="tb")
                          nc.vector.tensor_mul(tb, sfi[:, 0:48], tg)
                          cT = cpool.tile([128, 3 * BC], f32, tag="c")
                          nc.vector.tensor_add(cT, ta, tb)
                          tc2 = spool.tile([128, 48], f32, tag="tc2")
                          nc.scalar.activation(tc2, cT, AF.Tanh)
                          nc.vector.tensor_mul(h2blk[:, :, s, :], so, tc2)
                      nc.sync.dma_start(out=h2_d.ap()[:, blk], in_=h2blk)
                      h2prev = h2blk

            # ================= Phase C: logits + log_softmax =================
            with ExitStack() as pc:
              if 'c' in phases:
                  h2r = pc.enter_context(tc.tile_pool(name="h2r", bufs=3))
                  encr = pc.enter_context(tc.tile_pool(name="encr", bufs=3))
                  smp = pc.enter_context(tc.tile_pool(name="smp", bufs=4))
                  smc = pc.enter_context(tc.tile_pool(name="smc", bufs=6))
                  psc = pc.enter_context(tc.tile_pool(name="psC", bufs=4, space="PSUM"))

                  for ch in range(COLS // 128):
                      h2t = h2r.tile([128, 3, 128], bf16)
                      s0 = (ch % 2) * 8
                      nc.sync.dma_start(out=h2t,
                                        in_=h2_d.ap()[:, ch // 2, :, s0:s0 + 8, :])
                      enct = encr.tile([128, 6, 128], bf16)
                      nc.sync.dma_start(out=enct, in_=encTd.ap()[ch])
                      ps = psc.tile([128, LABEL], f32)
                      for k in range(3):
                          nc.tensor.matmul(ps, h2t[:, k, :], outWhT_sb[:, k, :],
                                           start=(k == 0), stop=False)
                      for fc in range(6):
                          nc.tensor.matmul(ps, enct[:, fc, :],
                                           outWeT_sb[:, fc, :],
                                           start=False, stop=(fc == 5))
                      lg = smp.tile([128, LABEL], f32, tag="lg")
                      nc.vector.tensor_copy(lg, ps)
                      if ch == 0:
                          nc.vector.memset(lg[0:BC, APP_ID:APP_ID + 1], -1e10)
                      mx = smc.tile([128, 1], f32, tag="mx")
                      nc.vector.tensor_reduce(mx, lg, mybir.AxisListType.X, ALU.max)
                      xm = smp.tile([128, LABEL], f32, tag="xm")
                      nc.vector.tensor_scalar(xm, lg, mx, None, ALU.subtract)
                      et = smp.tile([128, LABEL], f32, tag="et")
                      ssum = smc.tile([128, 1], f32, tag="ssum")
                      nc.scalar.activation(et, xm, AF.Exp, accum_out=ssum)
                      lns = smc.tile([128, 1], f32, tag="lns")
                      nc.scalar.activation(lns, ssum, AF.Ln)
                      res = smp.tile([128, LABEL], f32, tag="res")
                      nc.vector.tensor_scalar(res, xm, lns, None, ALU.subtract)
                      tc0 = ch * 8
                      nc.sync.dma_start(
                          out=out.ap().rearrange("b t l -> t b l")[tc0:tc0 + 8, :, :],
                          in_=res)

    nc.compile()
    return nc


def _host_prep(encoder_out, pos_embed_w, W_ih, W_hh, b_ih, b_hh,
               combine_W, combine_b, out_W, word_start, pos_ids):
    bf = ml_dtypes.bfloat16
    enc = np.asarray(encoder_out, dtype=np.float32)
    ws = np.asarray(word_start)
    pid = np.asarray(pos_ids)
    tgrid = np.arange(T)[:, None]
    valid = ws >= 0
    s = np.clip(ws, 0, None)
    ln = np.maximum(tgrid - s, 1)
    recipv = (valid / ln).astype(np.float32)
    t0 = (tgrid // 128) * 128
    sreld = (s - t0).astype(np.float32)
    srelc = (s - t0 + 8).astype(np.float32)

    shared = dict(
        combWT=np.ascontiguousarray(
            np.asarray(combine_W, np.float32).T).reshape(7, 128, HID).astype(bf),
        wihT=np.ascontiguousarray(
            np.asarray(W_ih, np.float32).T).reshape(6, 128, 4 * H).astype(bf),
        whhT=np.ascontiguousarray(
            np.asarray(W_hh, np.float32).T).reshape(3, 128, 4 * H).astype(bf),
        outWhT=np.ascontiguousarray(
            np.asarray(out_W, np.float32)[:, :H].T).reshape(3, 128, LABEL).astype(bf),
        outWeT=np.ascontiguousarray(
            np.asarray(out_W, np.float32)[:, H:].T).reshape(6, 128, LABEL).astype(bf),
        posw=np.asarray(pos_embed_w, np.float32).astype(bf),
        combb=np.asarray(combine_b, np.float32).reshape(6, 128),
        biassum=(np.asarray(b_ih, np.float32)
                 + np.asarray(b_hh, np.float32)).reshape(12, 128),
        mlt=(np.arange(128)[:, None] < np.arange(128)[None, :]
             ).astype(np.float32),
        iota=np.arange(128, dtype=np.float32),
        id128=np.eye(128, dtype=np.float32).astype(bf),
    )
    in_maps = []
    for c in range(NCORES):
        bs = slice(c * BC, (c + 1) * BC)
        m = dict(shared)
        enc_c = enc[bs].astype(bf)                       # [BC, T, 768]
        m["enc"] = np.ascontiguousarray(enc_c)
        # encTd[ch, p, fc, ts, b] = enc[b, ch*8+ts, fc*128+p]
        e = enc_c.reshape(BC, COLS // 128, 8, 6, 128)
        m["encTd"] = np.ascontiguousarray(
            e.transpose(1, 4, 3, 2, 0)).reshape(COLS // 128, 128, 6 * 8 * BC)
        m["sreld"] = np.ascontiguousarray(sreld[:, bs].T)
        m["srelc"] = np.ascontiguousarray(srelc[:, bs].T)
        m["recipv"] = np.ascontiguousarray(recipv[:, bs].T)
        m["pidcol"] = np.ascontiguousarray(
            pid[:, bs].astype(np.float32).reshape(-1))
        in_maps.append(m)
    return in_maps


def _get_compiled():
    global _COMPILED
    if _COMPILED is None:
        import os
        reps = int(os.environ.get("BK_REPS", "1"))
        phases = os.environ.get("BK_PHASES", "abc")
        _COMPILED = _build(reps=reps, phases=phases)
    return _COMPILED


def kernel(**inputs):
    from concourse.bass_utils import run_bass_kernel_spmd
    nc = _get_compiled()
    in_maps = _host_prep(**inputs)
    res = run_bass_kernel_spmd(nc, in_maps, list(range(NCORES)))
    outs = [res.results[c]["out"] for c in range(NCORES)]
    full = np.concatenate(outs, axis=0)           # [B, T, LABEL]
    return full.reshape(B * T, LABEL).astype(np.float32)
